# revision 3
# baseline (speedup 1.0000x reference)
"""Trainium2 Bass kernel for nn_DiffusionBlock (anisotropic diffusion step).

Math (per batch, channel image; s = tau*hx^2, hx = grad kernel tap):
  X[i,j] = u[i,j+1]-u[i,j] (0 at j=W-1),  Y[i,j] = u[i+1,j]-u[i,j] (0 at i=H-1)
  XP/YP  = edge-pad(X/Y) on the (H+2, W+2) grid
  F = sa*XP + sb*YP,  G = sb*XP + sc*YP          (sa/sb/sc = s-prescaled a/b/c)
  out[i,j] = u[i,j] + F[i+1,j+1]-F[i+1,j] + G[i+1,j+1]-G[i,j+1]

HBM traffic is the bottleneck (loads ~289 GB/s, stores ~170-200 GB/s,
measured): coefficients travel as fp8e4m3 (prescaled by s on the host) and
the kernel stores only the DELTA (out - u) as fp8e4m3; the host adds the
exact f32 u back. u travels bf16. Measured pure-DMA floor for this traffic
(9x [128,10252B] loads + 9x [126,2048B] stores): ~55.5 us.

Per-core layout (pure batch data-parallel, 1 batch item per core): row
tiles of R=126 output rows, both channels processed per tile. SBUF
partition q holds padded-grid row r0+q:
  U[q]     = u row clamp(r0+q-1)   [128, 2ch, 1024] bf16
  A/B/C[q] = s*a/b/c row r0+q      [128, 2ch, 1026] fp8
Per tile (both channels per instruction via 3D channel-strided APs):
  DVE : XTE = col-clamped X on [*,ch,1025]; products PA=Ab*XTE, PB1=Bb*YTE,
        PB2=Bb*XTE, PC=Cb*YTE (all bf16 SBUF -> 2x DVE mode)
  PE  : YTB[q,ch] = U[q+1]-U[q] via bidiagonal matmul into PSUM f32;
        after YTE/products consume it, 6 accumulating matmuls per chunk
        per channel overwrite YTB (start=True) with the delta;
        partition-shift matrices wsp/wsn/wg carry the +-1 (s prescaled).
  ACT : YTE = col-clamped bf16 copy of YTB; A upconvert fp8->bf16;
        OS = fp8 delta copy of the assembled PSUM.
  Pool: B, C upconverts fp8->bf16.
"""

import numpy as np

# Problem geometry (hardcoded per harness contract).
N_CORES = 8
N_CH = 2
H = 1024
W = 1024
R = 126       # output rows per tile
CHUNK = 512   # matmul free-dim chunk (= one PSUM bank of fp32)

_W_NAMES = ("wsp", "wsn", "wg", "my", "myf", "myl", "myfl")


def _host_weights(rt_last: int):
    """Constant PE weight matrices, packed [128, 7*128] bf16.

    matmul(out, lhsT, rhs): out[p, n] = sum_k lhsT[k, p] * rhs[k, n]
    """
    import ml_dtypes

    k = np.arange(128)[:, None]
    p = np.arange(128)[None, :]
    wsp = (k == p + 1).astype(np.float32)           # out[p] += x[p+1]
    wsn = -wsp                                      # out[p] -= x[p+1]
    wg = wsp - (k == p).astype(np.float32)          # out[p] += x[p+1]-x[p]
    my = wg.copy()                                  # YT[q] = U[q+1]-U[q]
    myf = my.copy()                                 # first tile: YT[0] = U[2]-U[1]
    myf[:, 0] = 0.0
    myf[2, 0] = 1.0
    myf[1, 0] = -1.0
    myl = my.copy()                                 # last tile: YT[rt] = 0
    myl[:, rt_last] = 0.0
    myfl = myf.copy()
    myfl[:, rt_last] = 0.0
    mats = {"wsp": wsp, "wsn": wsn, "wg": wg,
            "my": my, "myf": myf, "myl": myl, "myfl": myfl}
    return np.ascontiguousarray(
        np.concatenate([mats[n] for n in _W_NAMES], axis=1)
    ).astype(ml_dtypes.bfloat16)


def _build_nc(n_ch: int, h: int, w: int, r: int, chunk: int, reps: int = 1,
              mode: str = "full"):
    import concourse.bacc as bacc
    import concourse.mybir as mybir
    import concourse.tile as tile

    f32 = mybir.dt.float32
    bf16 = mybir.dt.bfloat16
    fp8 = mybir.dt.float8e4

    wp = w + 2  # 1026

    nc = bacc.Bacc()
    ud_d = nc.dram_tensor("ud", [h + 2, n_ch, w], bf16, kind="ExternalInput")
    a_d = nc.dram_tensor("a_s", [h + 2, n_ch, wp], fp8, kind="ExternalInput")
    b_d = nc.dram_tensor("b_s", [h + 2, n_ch, wp], fp8, kind="ExternalInput")
    c_d = nc.dram_tensor("c_s", [h + 2, n_ch, wp], fp8, kind="ExternalInput")
    wts_d = nc.dram_tensor(
        "wts", [128, len(_W_NAMES) * 128], bf16, kind="ExternalInput")
    out_d = nc.dram_tensor("out", [h, n_ch, w], fp8, kind="ExternalOutput")

    tiles = [(r0, min(r, h - r0)) for r0 in range(0, h, r)]
    if mode == "min":
        with tile.TileContext(nc) as tc:
            with tc.tile_pool(name="io", bufs=1) as io:
                t = io.tile([1, 16], bf16, tag="t")
                nc.sync.dma_start(t[0:1, :], ud_d[0:1, 0, 0:16])
                nc.sync.dma_start(out_d[0:1, 0, 0:16], t[0:1, 0:16])
        nc.compile()
        return nc

    do_xt = mode in ("full", "nope", "nodve", "nomm")
    do_yt = mode in ("full", "nope", "nodve")
    do_cvt = mode in ("full", "nope", "nodve", "nomm")
    do_dve = mode in ("full", "nope", "nomm")
    do_pe = mode in ("full", "nodve")
    do_act = mode != "dma"

    with tile.TileContext(nc) as tc:
        with (
            tc.tile_pool(name="wpool", bufs=1) as wpool,
            tc.tile_pool(name="io", bufs=3) as io,
            tc.tile_pool(name="cvt", bufs=2) as cvt,
            tc.tile_pool(name="tmp", bufs=2) as tmp,
            tc.tile_pool(name="psum", bufs=2, space="PSUM") as psum,
        ):
            # one DMA for all weights, then a tiny high-priority matmul so no
            # per-tile matmul ever waits on this DMA (S3_LW wait slots <= 2)
            w_all = wpool.tile([128, len(_W_NAMES) * 128], bf16, tag="w_all")
            nc.sync.dma_start(w_all[:], wts_d[:])
            wt = {
                n: w_all[:, i * 128 : (i + 1) * 128]
                for i, n in enumerate(_W_NAMES)
            }
            warm = psum.tile([1, 4], f32, tag="YTB")
            with tc.high_priority():
                nc.tensor.matmul(warm[0:1, 0:1], w_all[0:1, 0:1], w_all[0:1, 0:1])

            for _rep in range(reps):
              for r0, rt in tiles:
                first = r0 == 0
                last = r0 + rt == h
                ka = rt + 1      # A/B/C/XTE/YTE/product partitions
                ku = rt + 1 if last else rt + 2  # loaded U partitions
                kld = 128 if ka == 127 else ka   # dodge 127-partition DMAs
                # ---- loads: u (bf16) and prescaled a/b/c (fp8) ----
                U = io.tile([128, n_ch, w], bf16, tag="U")
                Af = io.tile([128, n_ch, wp], fp8, tag="Af")
                Bf = io.tile([128, n_ch, wp], fp8, tag="Bf")
                Cf = io.tile([128, n_ch, wp], fp8, tag="Cf")
                nc.sync.dma_start(U[0:kld, :, :], ud_d[r0 : r0 + kld, :, :])
                nc.sync.dma_start(Af[0:kld, :, :], a_d[r0 : r0 + kld, :, :])
                nc.sync.dma_start(Bf[0:kld, :, :], b_d[r0 : r0 + kld, :, :])
                nc.sync.dma_start(Cf[0:kld, :, :], c_d[r0 : r0 + kld, :, :])

                # ---- XTE (DVE): col-clamped x-diff, [*, ch, 1025] ----
                # XTE[q,c,s] = X[q,c,clip(s-1)];  X[q,c,j] = U[q,c,j+1]-U[q,c,j]
                XTE = tmp.tile([128, n_ch, w + 1], bf16, tag="XTE")
                if do_xt:
                    nc.vector.tensor_sub(
                        XTE[0:ka, :, 1:w],
                        U[0:ka, :, 1:w], U[0:ka, :, 0 : w - 1])
                    nc.vector.tensor_sub(
                        XTE[0:ka, :, 0:1], U[0:ka, :, 1:2], U[0:ka, :, 0:1])
                    nc.vector.memset(XTE[0:ka, :, w : w + 1], 0.0)

                # ---- upconverts fp8 -> bf16 (ACT: A, Pool: B and C) ----
                Ab = cvt.tile([128, n_ch, w + 1], bf16, tag="Ab")
                Bb = cvt.tile([128, n_ch, w + 1], bf16, tag="Bb")
                Cb = cvt.tile([128, n_ch, w + 1], bf16, tag="Cb")
                if do_cvt:
                    nc.scalar.copy(Ab[0:ka, :, :], Af[0:ka, :, 0 : w + 1])
                    nc.gpsimd.tensor_copy(Bb[0:ka, :, :], Bf[0:ka, :, 0 : w + 1])
                    nc.gpsimd.tensor_copy(
                        Cb[0:ka, :, 1 : w + 1], Cf[0:ka, :, 1 : w + 1])

                # ---- YTB (PE): partition-dim diff -> PSUM f32 ----
                YTB = psum.tile([128, n_ch, w], f32, tag="YTB")
                my = wt[{(0, 0): "my", (1, 0): "myf",
                         (0, 1): "myl", (1, 1): "myfl"}[(first, last)]]
                if do_yt:
                    for ch in range(n_ch):
                        for n0 in range(0, w, chunk):
                            nc.tensor.matmul(
                                YTB[0:ka, ch, n0 : n0 + chunk],
                                my[0:ku, 0:ka],
                                U[0:ku, ch, n0 : n0 + chunk],
                            )

                # ---- YTE (ACT): col-clamped bf16 copy of YTB ----
                YTE = tmp.tile([128, n_ch, w + 1], bf16, tag="YTE")
                if do_act and do_yt:
                    nc.scalar.copy(YTE[0:ka, :, 1 : w + 1], YTB[0:ka, :, 0:w])
                    nc.scalar.copy(YTE[0:ka, :, 0:1], YTB[0:ka, :, 0:1])
                elif do_dve:
                    nc.vector.memset(YTE[0:ka, :, 0:4], 0.0)

                # ---- products (DVE, all bf16 SBUF -> 2x) ----
                PA = tmp.tile([128, n_ch, w + 1], bf16, tag="PA")
                PB1 = tmp.tile([128, n_ch, w + 1], bf16, tag="PB1")
                PB2 = tmp.tile([128, n_ch, w], bf16, tag="PB2")
                PC = tmp.tile([128, n_ch, w], bf16, tag="PC")
                if do_dve:
                    nc.vector.tensor_mul(
                        PA[0:ka, :, :], Ab[0:ka, :, :], XTE[0:ka, :, :])
                    nc.vector.tensor_mul(
                        PB1[0:ka, :, :], Bb[0:ka, :, :], YTE[0:ka, :, :])
                    nc.vector.tensor_mul(
                        PB2[0:ka, :, :],
                        Bb[0:ka, :, 1 : w + 1], XTE[0:ka, :, 1 : w + 1])
                    nc.vector.tensor_mul(
                        PC[0:ka, :, :],
                        Cb[0:ka, :, 1 : w + 1], YTE[0:ka, :, 1 : w + 1])

                # ---- delta assembly (PE): overwrite YTB with the delta ----
                # delta[p,c,j] = PA[p+1,c,j+1] - PA[p+1,c,j]
                #             + PB1[p+1,c,j+1] - PB1[p+1,c,j]
                #             + PB2[p+1,c,j] - PB2[p,c,j]
                #             + PC[p+1,c,j] - PC[p,c,j]
                for ch in (range(n_ch) if do_pe else ()):
                    for n0 in range(0, w, chunk):
                        cw = min(chunk, w - n0)
                        o = YTB[0:rt, ch, n0 : n0 + cw]
                        mm = [
                            (wt["wsp"], PA[0:ka, ch, n0 + 1 : n0 + 1 + cw]),
                            (wt["wsn"], PA[0:ka, ch, n0 : n0 + cw]),
                            (wt["wsp"], PB1[0:ka, ch, n0 + 1 : n0 + 1 + cw]),
                            (wt["wsn"], PB1[0:ka, ch, n0 : n0 + cw]),
                            (wt["wg"], PB2[0:ka, ch, n0 : n0 + cw]),
                            (wt["wg"], PC[0:ka, ch, n0 : n0 + cw]),
                        ]
                        for i, (lhsT, rhs) in enumerate(mm):
                            nc.tensor.matmul(
                                o,
                                lhsT[0:ka, 0:rt],
                                rhs,
                                start=(i == 0),
                                stop=(i == len(mm) - 1),
                            )

                # ---- delta PSUM -> SBUF fp8 (ACT), store ----
                OS = tmp.tile([128, n_ch, w], fp8, tag="OS")
                if do_act:
                    nc.scalar.copy(OS[0:rt, :, :], YTB[0:rt, :, :])
                else:
                    nc.vector.memset(OS[0:1, 0, 0:4], 0)
                if do_act and not do_pe and not do_yt:
                    nc.vector.memset(YTB[0:1, 0, 0:4], 0.0)
                nc.sync.dma_start(out_d[r0 : r0 + rt, :, :], OS[0:rt, :, :])

    nc.compile()
    return nc


def _cast_inputs(u, a, b, c, s):
    """Host packing: ud[r, ch, :] = bf16 u row clamp(r-1);
    {a,b,c}_s[r, ch, :] = fp8e4m3 of (s * field[ch, r, :])."""
    import ml_dtypes

    bf = ml_dtypes.bfloat16
    f8 = ml_dtypes.float8_e4m3fn
    u_np = np.asarray(u, dtype=np.float32)
    n, ch, h, w = u_np.shape
    rows = np.clip(np.arange(h + 2) - 1, 0, h - 1)
    ud = np.ascontiguousarray(
        u_np[:, :, rows, :].transpose(0, 2, 1, 3).astype(bf))  # [n, h+2, ch, w]
    sf = np.float32(s)
    packed = [
        np.ascontiguousarray(
            (np.asarray(t, dtype=np.float32) * sf)
            .transpose(0, 2, 1, 3).astype(f8))  # [n, h+2, ch, w+2]
        for t in (a, b, c)
    ]
    return ud, packed[0], packed[1], packed[2]


def kernel(u, a, b, c, tau, grad_x, grad_y):
    from concourse.bass_utils import run_bass_kernel_spmd

    u_np = np.asarray(u, dtype=np.float32)
    hx = float(np.asarray(grad_x)[0, 0, 1, 2])
    s = float(np.asarray(tau)) * hx * hx
    ud, a_s, b_s, c_s = _cast_inputs(u, a, b, c, s)
    rt_last = H % R if H % R else R
    wts = _host_weights(rt_last)

    nc = _build_nc(N_CH, H, W, R, CHUNK)
    in_maps = [
        {"ud": ud[k], "a_s": a_s[k], "b_s": b_s[k], "c_s": c_s[k], "wts": wts}
        for k in range(N_CORES)
    ]
    res = run_bass_kernel_spmd(nc, in_maps, list(range(N_CORES)))
    # delta [h, ch, w] fp8 -> out = u + delta
    delta = np.stack(
        [
            res.results[k]["out"].astype(np.float32).transpose(1, 0, 2)
            for k in range(N_CORES)
        ],
        axis=0,
    )
    return u_np + delta


# revision 5
# speedup vs baseline: 2.1100x; 2.1100x over previous
"""Trainium2 Bass kernel for nn_DiffusionBlock (anisotropic diffusion step).

Math (per batch, channel image; s = tau*hx^2, hx = grad kernel tap):
  X[i,j] = u[i,j+1]-u[i,j] (0 at j=W-1),  Y[i,j] = u[i+1,j]-u[i,j] (0 at i=H-1)
  XP/YP  = edge-pad(X/Y) on the (H+2, W+2) grid
  F = sa*XP + sb*YP,  G = sb*XP + sc*YP          (sa/sb/sc = s-prescaled a/b/c)
  out[i,j] = u[i,j] + F[i+1,j+1]-F[i+1,j] + G[i+1,j+1]-G[i,j+1]

HBM traffic is the bottleneck (loads ~289 GB/s, stores ~170-200 GB/s,
measured): coefficients travel as fp8e4m3 (prescaled by s on the host) and
are upconverted to bf16 INSIDE the load DMA (gpsimd/SWDGE casting dma_start,
measured ~= plain fp8 DMA cost); the kernel stores only the DELTA (out - u)
as fp8e4m3 and the host adds the exact f32 u back. u travels bf16.
Measured pure-DMA floor for this traffic mix: ~55 us.

Per-core layout (pure batch data-parallel, 1 batch item per core): row
tiles of R=126 output rows per channel (18 channel-tiles). SBUF partition q
holds padded-grid row r0+q; all engine ops use plain 2D APs (3D
channel-strided APs measured 2-4x slower). Per channel-tile:
  DVE : XTE[q,s] = X[q,clip(s-1)] (col-clamped, [*,1025]); products
        PA=A*XTE, PB1=B*YTE, PB2=B*XTE', PC=C*YTE' (all bf16 SBUF -> 2x)
  PE  : YTB[q] = U[q+1]-U[q] (bidiagonal matmul, PSUM f32); after YTE is
        taken, 6 accumulating matmuls per 512-chunk overwrite YTB
        (start=True) with the delta; wsp/wsn/wg carry the +-1 signs.
  ACT : YTE = col-clamped bf16 copy of YTB; OS = fp8 delta copy of YTB.
"""

import numpy as np

# Problem geometry (hardcoded per harness contract).
N_CORES = 8
N_CH = 2
H = 1024
W = 1024
R = 126       # output rows per tile
CHUNK = 512   # matmul free-dim chunk (= one PSUM bank of fp32)

_W_NAMES = ("wsp", "wsn", "wg", "my", "myf", "myl", "myfl")


def _host_weights(rt_last: int):
    """Constant PE weight matrices, packed [128, 7*128] bf16.

    matmul(out, lhsT, rhs): out[p, n] = sum_k lhsT[k, p] * rhs[k, n]
    """
    import ml_dtypes

    k = np.arange(128)[:, None]
    p = np.arange(128)[None, :]
    wsp = (k == p + 1).astype(np.float32)           # out[p] += x[p+1]
    wsn = -wsp                                      # out[p] -= x[p+1]
    wg = wsp - (k == p).astype(np.float32)          # out[p] += x[p+1]-x[p]
    my = wg.copy()                                  # YT[q] = U[q+1]-U[q]
    myf = my.copy()                                 # first tile: YT[0] = U[2]-U[1]
    myf[:, 0] = 0.0
    myf[2, 0] = 1.0
    myf[1, 0] = -1.0
    myl = my.copy()                                 # last tile: YT[rt] = 0
    myl[:, rt_last] = 0.0
    myfl = myf.copy()
    myfl[:, rt_last] = 0.0
    mats = {"wsp": wsp, "wsn": wsn, "wg": wg,
            "my": my, "myf": myf, "myl": myl, "myfl": myfl}
    return np.ascontiguousarray(
        np.concatenate([mats[n] for n in _W_NAMES], axis=1)
    ).astype(ml_dtypes.bfloat16)


def _build_nc(n_ch: int, h: int, w: int, r: int, chunk: int, reps: int = 1,
              mode: str = "full"):
    import concourse.bacc as bacc
    import concourse.mybir as mybir
    import concourse.tile as tile

    f32 = mybir.dt.float32
    bf16 = mybir.dt.bfloat16
    fp8 = mybir.dt.float8e4

    wp = w + 2  # 1026

    nc = bacc.Bacc()
    ud_d = nc.dram_tensor("ud", [n_ch, h + 2, w], bf16, kind="ExternalInput")
    abc_d = nc.dram_tensor(
        "abc_s", [n_ch, h + 2, 3 * wp], fp8, kind="ExternalInput")
    wts_d = nc.dram_tensor(
        "wts", [128, len(_W_NAMES) * 128], bf16, kind="ExternalInput")
    out_d = nc.dram_tensor("out", [n_ch, h, w], fp8, kind="ExternalOutput")

    tiles = [(r0, min(r, h - r0)) for r0 in range(0, h, r)]
    if mode == "min":
        with tile.TileContext(nc) as tc:
            with tc.tile_pool(name="io", bufs=1) as io:
                t = io.tile([1, 16], bf16, tag="t")
                nc.sync.dma_start(t[0:1, :], ud_d[0, 0:1, 0:16])
                nc.sync.dma_start(out_d[0, 0:1, 0:16], t[0:1, 0:16])
        nc.compile()
        return nc

    do_xt = mode in ("full", "nope", "nodve", "nomm")
    do_yt = mode in ("full", "nope", "nodve")
    do_dve = mode in ("full", "nope", "nomm")
    do_pe = mode in ("full", "nodve")
    do_act = mode != "dma"

    with tile.TileContext(nc) as tc:
        with (
            tc.tile_pool(name="wpool", bufs=1) as wpool,
            tc.tile_pool(name="io", bufs=4) as io,
            tc.tile_pool(name="tmp", bufs=3) as tmp,
            tc.tile_pool(name="psum", bufs=4, space="PSUM") as psum,
        ):
            # one DMA for all weights, then a tiny high-priority matmul so no
            # per-tile matmul ever waits on this DMA (S3_LW wait slots <= 2)
            w_all = wpool.tile([128, len(_W_NAMES) * 128], bf16, tag="w_all")
            nc.sync.dma_start(w_all[:], wts_d[:])
            wt = {
                n: w_all[:, i * 128 : (i + 1) * 128]
                for i, n in enumerate(_W_NAMES)
            }
            warm = psum.tile([1, 4], f32, tag="YTB")
            with tc.high_priority():
                nc.tensor.matmul(warm[0:1, 0:1], w_all[0:1, 0:1], w_all[0:1, 0:1])

            for _rep in range(reps):
              for ch in range(n_ch):
                for r0, rt in tiles:
                    first = r0 == 0
                    last = r0 + rt == h
                    ka = rt + 1      # A/B/C/XTE/YTE/product partitions
                    ku = rt + 1 if last else rt + 2  # loaded U partitions
                    kld = 128 if ka == 127 else ka   # dodge 127-part DMAs
                    # ---- loads: u (bf16, HWDGE) and prescaled a|b|c
                    #      (fp8 -> bf16 casting SWDGE dma) ----
                    U = io.tile([128, w], bf16, tag="U")
                    ABC = io.tile([128, 3 * wp], bf16, tag="ABC")
                    nc.sync.dma_start(
                        U[0:kld, :], ud_d[ch, r0 : r0 + kld, :])
                    nc.gpsimd.dma_start(
                        ABC[0:kld, :], abc_d[ch, r0 : r0 + kld, :])
                    A = ABC[:, 0:wp]
                    B = ABC[:, wp : 2 * wp]
                    C = ABC[:, 2 * wp : 3 * wp]

                    # ---- XTE (DVE): col-clamped x-diff, [*, 1025] ----
                    # XTE[q,s] = X[q,clip(s-1)];  X[q,j] = U[q,j+1]-U[q,j]
                    XTE = tmp.tile([128, w + 1], bf16, tag="XTE")
                    if do_xt:
                        nc.vector.tensor_sub(
                            XTE[0:ka, 1:w], U[0:ka, 1:w], U[0:ka, 0 : w - 1])
                        nc.vector.tensor_sub(
                            XTE[0:ka, 0:1], U[0:ka, 1:2], U[0:ka, 0:1])
                        nc.vector.memset(XTE[0:ka, w : w + 1], 0.0)

                    # ---- YTB (PE): partition-dim diff -> PSUM f32 ----
                    YTB = psum.tile([128, w], f32, tag="YTB")
                    my = wt[{(0, 0): "my", (1, 0): "myf",
                             (0, 1): "myl", (1, 1): "myfl"}[(first, last)]]
                    if do_yt:
                        for n0 in range(0, w, chunk):
                            nc.tensor.matmul(
                                YTB[0:ka, n0 : n0 + chunk],
                                my[0:ku, 0:ka],
                                U[0:ku, n0 : n0 + chunk],
                            )

                    # ---- YTE (ACT): col-clamped bf16 copy of YTB ----
                    YTE = tmp.tile([128, w + 1], bf16, tag="YTE")
                    if do_act and do_yt:
                        nc.scalar.copy(YTE[0:ka, 1 : w + 1], YTB[0:ka, 0:w])
                        nc.scalar.copy(YTE[0:ka, 0:1], YTB[0:ka, 0:1])
                    elif do_dve:
                        nc.vector.memset(YTE[0:ka, :], 0.0)

                    # ---- products (DVE, all bf16 SBUF -> 2x) ----
                    PA = tmp.tile([128, w + 1], bf16, tag="PA")
                    PB1 = tmp.tile([128, w + 1], bf16, tag="PB1")
                    PB2 = tmp.tile([128, w], bf16, tag="PB2")
                    PC = tmp.tile([128, w], bf16, tag="PC")
                    if do_dve:
                        nc.vector.tensor_mul(
                            PA[0:ka, :], A[0:ka, 0 : w + 1], XTE[0:ka, :])
                        nc.vector.tensor_mul(
                            PB1[0:ka, :], B[0:ka, 0 : w + 1], YTE[0:ka, :])
                        nc.vector.tensor_mul(
                            PB2[0:ka, :],
                            B[0:ka, 1 : w + 1], XTE[0:ka, 1 : w + 1])
                        nc.vector.tensor_mul(
                            PC[0:ka, :],
                            C[0:ka, 1 : w + 1], YTE[0:ka, 1 : w + 1])

                    # ---- delta assembly (PE): overwrite YTB with delta ----
                    # delta[p,j] = PA[p+1,j+1] - PA[p+1,j]
                    #            + PB1[p+1,j+1] - PB1[p+1,j]
                    #            + PB2[p+1,j] - PB2[p,j] + PC[p+1,j] - PC[p,j]
                    for n0 in (range(0, w, chunk) if do_pe else ()):
                        cw = min(chunk, w - n0)
                        o = YTB[0:rt, n0 : n0 + cw]
                        mm = [
                            (wt["wsp"], PA[0:ka, n0 + 1 : n0 + 1 + cw]),
                            (wt["wsn"], PA[0:ka, n0 : n0 + cw]),
                            (wt["wsp"], PB1[0:ka, n0 + 1 : n0 + 1 + cw]),
                            (wt["wsn"], PB1[0:ka, n0 : n0 + cw]),
                            (wt["wg"], PB2[0:ka, n0 : n0 + cw]),
                            (wt["wg"], PC[0:ka, n0 : n0 + cw]),
                        ]
                        for i, (lhsT, rhs) in enumerate(mm):
                            nc.tensor.matmul(
                                o,
                                lhsT[0:ka, 0:rt],
                                rhs,
                                start=(i == 0),
                                stop=(i == len(mm) - 1),
                            )

                    # ---- delta PSUM -> SBUF fp8 (ACT), store ----
                    OS = tmp.tile([128, w], fp8, tag="OS")
                    if do_act:
                        nc.scalar.copy(OS[0:rt, :], YTB[0:rt, :])
                    else:
                        nc.vector.memset(OS[0:1, 0:4], 0)
                    if do_act and not do_pe and not do_yt:
                        nc.vector.memset(YTB[0:1, 0:4], 0.0)
                    if do_pe and not do_dve:
                        for _t in (PA, PB1, PB2, PC):
                            nc.vector.memset(_t[0:1, 0:4], 0.0)
                    nc.sync.dma_start(out_d[ch, r0 : r0 + rt, :], OS[0:rt, :])

    nc.compile()
    return nc


def _cast_inputs(u, a, b, c, s):
    """Host packing: ud[ch, r, :] = bf16 u row clamp(r-1);
    abc_s[ch, r, :] = fp8e4m3 of s*[a|b|c][ch, r, :] concatenated."""
    import ml_dtypes

    bf = ml_dtypes.bfloat16
    f8 = ml_dtypes.float8_e4m3fn
    u_np = np.asarray(u, dtype=np.float32)
    n, ch, h, w = u_np.shape
    rows = np.clip(np.arange(h + 2) - 1, 0, h - 1)
    ud = np.ascontiguousarray(u_np[:, :, rows, :].astype(bf))  # [n, ch, h+2, w]
    sf = np.float32(s)
    abc = np.concatenate(
        [np.asarray(t, dtype=np.float32) * sf for t in (a, b, c)], axis=3
    )  # [n, ch, h+2, 3*(w+2)]
    abc_s = np.ascontiguousarray(abc.astype(f8))
    return ud, abc_s


def kernel(u, a, b, c, tau, grad_x, grad_y):
    from concourse.bass_utils import run_bass_kernel_spmd

    u_np = np.asarray(u, dtype=np.float32)
    hx = float(np.asarray(grad_x)[0, 0, 1, 2])
    s = float(np.asarray(tau)) * hx * hx
    ud, abc_s = _cast_inputs(u, a, b, c, s)
    rt_last = H % R if H % R else R
    wts = _host_weights(rt_last)

    nc = _build_nc(N_CH, H, W, R, CHUNK)
    in_maps = [
        {"ud": ud[k], "abc_s": abc_s[k], "wts": wts}
        for k in range(N_CORES)
    ]
    res = run_bass_kernel_spmd(nc, in_maps, list(range(N_CORES)))
    # delta [ch, h, w] fp8 -> out = u + delta
    delta = np.stack(
        [res.results[k]["out"].astype(np.float32) for k in range(N_CORES)],
        axis=0,
    )
    return u_np + delta
